# revision 1
# baseline (speedup 1.0000x reference)
"""Trainium2 Bass kernel for the N^3 triplet descriptor (gnn_message_passing).

Strategy: the reference's O(N^3) angular sum factorizes exactly via the
Legendre addition theorem into O(N^2) per-pair vector moments:

  P0 term: (sum_j w_j)^2
  P1 term: |sum_j w_j u_j|^2                  (u = unit displacement)
  P2 term: 1.5*|sum_j w_j u_j u_j^T|_F^2 - 0.5*(sum_j w_j)^2

with w_j = fc(r_ij) * r_ij^n.  Each device accumulates 36 pair moments per
central atom (9 radial powers, 9 S1 components, 9+9 symmetric S2
components); the tiny nonlinear combine runs on host after gathering.

All per-pair weights belong to one family e_k = fc * r^(k-2), k=0..10:
radial moments reduce e_2..e_10; S1 weights are e_1..e_3; S2 weights are
e_0..e_2 — a single tile built with 5 strided DVE ops serves everything.

Sharding: 8 cores = 2 i-blocks (96 rows on partitions) x 4 j-chunks (48
neighbors on the free axis). Cross-j-chunk partials are summed on host.

Implementation: raw Bass (no Tile framework) with per-engine semaphore
chains. Heavy lifting stays on the DVE; GpSimd only computes geometry
that overlaps contiguous DVE phases (concurrent GpSimd tensor ops slow
strided DVE ops ~4x via SBUF port contention — measured). The single ACT
table (abs_reciprocal_sqrt_and_small) provides 1/r = 1/sqrt(r^2+eps) and
r = r2 * rinv; fc is a degree-5 polynomial in r^2 with an exact
(r^2 < RC^2) cutoff mask. Input/output DMAs are split across the sync
and scalar HWDGE queues for parallel descriptor generation.
"""

import numpy as np

import concourse.bass as bass
import concourse.bacc as bacc
from concourse import mybir
from concourse.bass_utils import run_bass_kernel_spmd

F32 = mybir.dt.float32
ALU = mybir.AluOpType
ACT = mybir.ActivationFunctionType

N = 192
NI = 96          # i rows per core (partition dim)
NJ = 48          # j neighbors per core (free dim)
NIB = 2          # i blocks
NJC = 4          # j chunks
BOX_L = 20.0
RC = 5.0
FC_DEG = 5   # deg-5 fit err 8.7e-7; amplified by r^8 near cutoff stays ~5e-5
R2_EPS = 1e-12

# fc(w) = 0.5*(1+cos(pi*sqrt(w)/RC)) as poly in w = r^2, w in [0, RC^2]
_FC_W = np.linspace(0, RC * RC, 20001)
_FC_Y = 0.5 * (1 + np.cos(np.pi * np.sqrt(_FC_W) / RC))
_FC_C = (
    np.polynomial.chebyshev.Chebyshev.fit(_FC_W, _FC_Y, FC_DEG, domain=[0, RC * RC])
    .convert(kind=np.polynomial.Polynomial)
    .coef.astype(np.float64)
)

_cached = {}


def _v(ap, off, dims):
    """Custom free-dim view of an SBUF tile AP: keep partition dim, replace
    free dims, shift offset by `off` elements."""
    return bass.AP(ap.tensor, ap.offset + off, [list(ap.ap[0])] + [list(d) for d in dims])


def build_nc():
    # Suppress the Bass.__init__ const-pool preamble (4 gpsimd memsets + an
    # all-engine barrier, ~0.8us of kernel startup): this kernel uses no
    # built-in const APs — its only const (c_eps) is set inside the block.
    _orig_barrier = bass.Bass.all_engine_barrier
    _orig_memset = bass.BassSharedVectorInterface.memset
    bass.Bass.all_engine_barrier = lambda self: None
    bass.BassSharedVectorInterface.memset = lambda self, ap, v: None
    try:
        nc = bacc.Bacc(
            "TRN2",
            target_bir_lowering=False,
            debug=False,
            enable_asserts=True,
            num_devices=NIB * NJC,
        )
    finally:
        bass.Bass.all_engine_barrier = _orig_barrier
        bass.BassSharedVectorInterface.memset = _orig_memset
    rji_d = nc.dram_tensor("rji", [NI, 160], F32, kind="ExternalInput").ap()
    out_d = nc.dram_tensor("out", [NI, 36], F32, kind="ExternalOutput").ap()

    rji = nc.alloc_sbuf_tensor("rji_s", [NI, 160], F32).ap()
    dxr = nc.alloc_sbuf_tensor("dxr", [NI, 144], F32).ap()
    # geo = [dx | sq | poff]; products read sq|poff and dx contiguously
    geo = nc.alloc_sbuf_tensor("geo", [NI, 432], F32).ap()
    # rvp = [rinv | r | r2 | r4]
    rvp = nc.alloc_sbuf_tensor("rvp", [NI, 192], F32).ap()
    m25 = nc.alloc_sbuf_tensor("m25", [NI, NJ], F32).ap()
    yh = nc.alloc_sbuf_tensor("yh", [NI, NJ], F32).ap()
    yh144 = nc.alloc_sbuf_tensor("yh144", [NI, 144], F32).ap()
    # wx blocks k=0..10: fc * r^(k-2)
    wx = nc.alloc_sbuf_tensor("wx", [NI, 11 * NJ], F32).ap()
    big3 = nc.alloc_sbuf_tensor("big3", [NI, 1296], F32).ap()  # T | bigd | bigo
    sg = nc.alloc_sbuf_tensor("sg", [NI, 36], F32).ap()
    scr = nc.alloc_sbuf_tensor("scr", [1, 8], F32).ap()
    # const for the ACT bias (set by GpSimd at program start)
    c_eps = nc.alloc_sbuf_tensor("c_eps", [128, 1], F32).ap()
    nc.const_aps.aps[(F32, R2_EPS)] = c_eps

    dsem = nc.alloc_semaphore("dsem")
    vq = nc.alloc_semaphore("vq")      # DVE instruction counter
    sqm = nc.alloc_semaphore("sqm")    # ACT instruction counter
    gq = nc.alloc_semaphore("gq")      # GpSimd instruction counter

    dx = geo[:, 0:144]
    sq_t = geo[:, 144:288]
    poff = geo[:, 288:432]
    rinv = rvp[:, 0:NJ]
    r = rvp[:, NJ:2 * NJ]
    r2 = rvp[:, 2 * NJ:3 * NJ]
    r4 = rvp[:, 3 * NJ:4 * NJ]
    fc = wx[:, 2 * NJ:3 * NJ]          # e2 = fc * r^0

    rj3 = rji[:, 0:144].rearrange("p (d j) -> p d j", d=3)
    ri3 = rji[:, 144:147].unsqueeze(-1).broadcast_to((NI, 3, NJ))
    dxr3 = dxr.rearrange("p (d j) -> p d j", d=3)

    c = [float(x) for x in _FC_C]

    # cross-engine wait points (per-engine instruction-counter values)
    VQ_DX = 5                  # dx ready
    VQ_R2 = 7                  # r2 ready
    VQ_QR = 16 + FC_DEG        # radial moments in sg
    VQ_REDA = 20 + FC_DEG      # S1 + S2diag moments in sg
    VQ_ALL = 21 + FC_DEG       # sg complete
    SQ_RINV = 2                # rinv ready
    GQ_EPS = 1                 # c_eps const set
    GQ_GEO = 4                 # poff + r4 ready

    with nc.Block() as block:

        @block.sync
        def _(sync):
            sync.dma_start(rji[:, 0:80], rji_d[:, 0:80]).then_inc(dsem, 16)
            sync.wait_ge(vq, VQ_QR)
            sync.dma_start(out_d[:, 0:9], sg[:, 0:9], single_packet=True).then_inc(dsem, 16)
            sync.wait_ge(vq, VQ_REDA)
            sync.dma_start(out_d[:, 9:27], sg[:, 9:27], single_packet=True).then_inc(dsem, 16)
            sync.wait_ge(dsem, 80)

        @block.scalar
        def _(scalar):
            sn = [0]

            def S(inst):
                # same-engine ordering chain (TRN2 engines pipeline;
                # RAW hazards need explicit sems — free at runtime)
                if sn[0] > 0:
                    inst._wait_ge(sqm, sn[0])
                inst.then_inc(sqm, 1)
                sn[0] += 1
                return inst

            # second half of the input DMA on the scalar HWDGE queue —
            # parallel descriptor-gen with sync's first half
            scalar.dma_start(rji[:, 80:160], rji_d[:, 80:160]).then_inc(dsem, 16)
            # dummy activation on the (just-memset) c_eps tile: pulls the
            # single ACT table load (abs_reciprocal_sqrt_and_small) to t=0,
            # overlapped with the input DMA + DVE distance math
            scalar.wait_ge(gq, GQ_EPS)
            S(scalar.activation(
                scr[0:1, 0:1], c_eps[0:1, :], ACT.Abs_reciprocal_sqrt,
                bias=R2_EPS))
            scalar.wait_ge(vq, VQ_R2)
            # rinv = 1/sqrt(r2 + eps); r recovered on DVE as r2 * rinv
            S(scalar.activation(rinv, r2, ACT.Abs_reciprocal_sqrt, bias=R2_EPS))
            assert sn[0] == SQ_RINV
            # final 9-col out-DMA from the idle Scalar HWDGE queue: its
            # descriptor-gen runs parallel to sync's, shortening the tail
            scalar.wait_ge(vq, VQ_ALL)
            scalar.dma_start(out_d[:, 27:36], sg[:, 27:36], single_packet=True).then_inc(dsem, 16)

        @block.gpsimd
        def _(gpsimd):
            gn = [0]

            def G(inst):
                if gn[0] > 0:
                    inst._wait_ge(gq, gn[0])
                inst.then_inc(gq, 1)
                gn[0] += 1
                return inst

            G(gpsimd.memset(c_eps, R2_EPS))
            # off-critical-path geometry on GpSimd; scheduled against DVE
            # phases with contiguous APs (strided-AP DVE phases suffer ~4x
            # from GpSimd SBUF port contention — measured)
            gpsimd.wait_ge(vq, VQ_DX)
            G(gpsimd.tensor_tensor(
                poff[:, 0:96], dx[:, 0:96], dx[:, 48:144], op=ALU.mult))
            G(gpsimd.tensor_tensor(
                poff[:, 96:144], dx[:, 0:NJ], dx[:, 96:144], op=ALU.mult))
            gpsimd.wait_ge(vq, VQ_R2)
            G(gpsimd.tensor_tensor(r4, r2, r2, op=ALU.mult))
            assert gn[0] == GQ_GEO

        @block.vector
        def _(vector):
            vn = [0]

            def V(inst):
                if vn[0] > 0:
                    inst._wait_ge(vq, vn[0])
                inst.then_inc(vq, 1)
                vn[0] += 1
                return inst

            vector.wait_ge(dsem, 32)
            V(vector.tensor_tensor(dxr3, rj3, ri3, op=ALU.subtract))
            # minimum image (box = BOX_L * I): dx -= L*(dxr>L/2); dx += L*(dxr<-L/2)
            V(vector.tensor_scalar(
                yh144, dxr, BOX_L / 2, BOX_L, op0=ALU.is_gt, op1=ALU.mult))
            V(vector.tensor_tensor(dx, dxr, yh144, op=ALU.subtract))
            V(vector.tensor_scalar(
                yh144, dxr, -BOX_L / 2, BOX_L, op0=ALU.is_lt, op1=ALU.mult))
            V(vector.tensor_tensor(dx, dx, yh144, op=ALU.add))
            V(vector.tensor_tensor(sq_t, dx, dx, op=ALU.mult))
            V(vector.reduce_sum(
                r2, sq_t.rearrange("p (d j) -> p j d", d=3),
                axis=mybir.AxisListType.X,
            ))
            assert vn[0] == VQ_R2
            # fc = poly(r2) * (r2 < RC^2), Horner on DVE
            V(vector.tensor_scalar(m25, r2, RC * RC, None, op0=ALU.is_lt))
            V(vector.tensor_scalar(yh, r2, c[FC_DEG], None, op0=ALU.mult))
            for k in range(FC_DEG - 1, 0, -1):
                V(vector.scalar_tensor_tensor(
                    yh, yh, c[k], r2, op0=ALU.add, op1=ALU.mult))
            V(vector.scalar_tensor_tensor(
                fc, yh, c[0], m25, op0=ALU.add, op1=ALU.mult))
            # weight family e_k = fc * r^(k-2) via strided block multiplies
            vector.wait_ge(sqm, SQ_RINV)
            V(vector.tensor_tensor(r, r2, rinv, op=ALU.mult))
            # [e1|e3] = fc * [rinv|r]
            V(vector.tensor_tensor(
                _v(wx, NJ, [[2 * NJ, 2], [1, NJ]]),
                _v(wx, 2 * NJ, [[0, 2], [1, NJ]]),
                _v(rvp, 0, [[NJ, 2], [1, NJ]]),
                op=ALU.mult))
            V(vector.tensor_tensor(wx[:, 0:NJ], wx[:, NJ:2 * NJ], rinv, op=ALU.mult))
            # [e4|e5] = [e2|e3] * r2
            V(vector.tensor_tensor(
                _v(wx, 4 * NJ, [[NJ, 2], [1, NJ]]),
                _v(wx, 2 * NJ, [[NJ, 2], [1, NJ]]),
                _v(rvp, 2 * NJ, [[0, 2], [1, NJ]]),
                op=ALU.mult))
            # [e6..e9] = [e2..e5] * r4  (r4 from GpSimd)
            vector.wait_ge(gq, GQ_GEO)
            V(vector.tensor_tensor(
                _v(wx, 6 * NJ, [[NJ, 4], [1, NJ]]),
                _v(wx, 2 * NJ, [[NJ, 4], [1, NJ]]),
                _v(rvp, 3 * NJ, [[0, 4], [1, NJ]]),
                op=ALU.mult))
            V(vector.tensor_tensor(
                wx[:, 10 * NJ:11 * NJ], wx[:, 6 * NJ:7 * NJ], r4, op=ALU.mult))
            # radial moments: q_r[k] = sum_j e_{k+2}
            V(vector.reduce_sum(
                sg[:, 0:9], _v(wx, 2 * NJ, [[NJ, 9], [1, NJ]]),
                axis=mybir.AxisListType.X,
            ))
            assert vn[0] == VQ_QR
            # S1 products: T[n,d] = e_{n+1} * dx_d -> big3[0:432]
            V(vector.tensor_tensor(
                _v(big3, 0, [[144, 3], [NJ, 3], [1, NJ]]),
                _v(wx, NJ, [[NJ, 3], [0, 3], [1, NJ]]),
                _v(geo, 0, [[0, 3], [NJ, 3], [1, NJ]]),
                op=ALU.mult))
            # S2 products: diag[n,d] = e_n * sq; off[n,m] = e_n * poff
            V(vector.tensor_tensor(
                _v(big3, 432, [[144, 3], [NJ, 3], [1, NJ]]),
                _v(wx, 0, [[NJ, 3], [0, 3], [1, NJ]]),
                _v(geo, 144, [[0, 3], [NJ, 3], [1, NJ]]),
                op=ALU.mult))
            V(vector.tensor_tensor(
                _v(big3, 864, [[144, 3], [NJ, 3], [1, NJ]]),
                _v(wx, 0, [[NJ, 3], [0, 3], [1, NJ]]),
                _v(geo, 288, [[0, 3], [NJ, 3], [1, NJ]]),
                op=ALU.mult))
            # merged reduce S1 + S2diag; S2off reduce last (small final
            # inst: the DVE tail pipe-DRAIN costs ~its duration again)
            V(vector.reduce_sum(
                sg[:, 9:27], _v(big3, 0, [[NJ, 18], [1, NJ]]),
                axis=mybir.AxisListType.X,
            ))
            assert vn[0] == VQ_REDA
            V(vector.reduce_sum(
                sg[:, 27:36], _v(big3, 864, [[NJ, 9], [1, NJ]]),
                axis=mybir.AxisListType.X,
            ))
            assert vn[0] == VQ_ALL, vn[0]

    nc.compile()
    return nc


def host_prep(R):
    """Per-core input arrays: [96, 160] = [RjT replicated | Ri | pad]."""
    R = np.ascontiguousarray(R, np.float32)
    in_maps = []
    for core in range(NIB * NJC):
        ib, jc = divmod(core, NJC)
        rji = np.zeros((NI, 160), np.float32)
        rj = R[jc * NJ:(jc + 1) * NJ, :]              # [48, 3]
        rji[:, 0:144] = rj.T.reshape(1, 144)          # d-major, replicated
        rji[:, 144:147] = R[ib * NI:(ib + 1) * NI, :]
        in_maps.append({"rji": rji})
    return in_maps


def host_combine(partials):
    """partials: list of 8 [96,36] arrays (core order). Returns [192,18]."""
    sums = np.zeros((N, 36), np.float64)
    for core, p in enumerate(partials):
        ib = core // NJC
        sums[ib * NI:(ib + 1) * NI] += p.astype(np.float64)
    sums = sums.astype(np.float32)
    q_r = sums[:, 0:9].copy()
    q_r[:, 0] -= 1.0                                  # remove j==i self term
    s0 = q_r[:, 0:3]                                  # [N,3] n=0..2
    s1 = sums[:, 9:18].reshape(N, 3, 3)               # [N,n,d]
    s2d = sums[:, 18:27].reshape(N, 3, 3)             # [N,n,d] diagonal
    s2o = sums[:, 27:36].reshape(N, 3, 3)             # [N,n,m] off-diagonal
    ang = np.empty((N, 3, 3), np.float32)
    ang[:, :, 0] = s0 * s0
    ang[:, :, 1] = (s1 * s1).sum(-1)
    fro2 = (s2d * s2d).sum(-1) + 2.0 * (s2o * s2o).sum(-1)
    ang[:, :, 2] = 1.5 * fro2 - 0.5 * s0 * s0
    return np.concatenate([q_r, ang.reshape(N, 9)], axis=-1)


def _get_nc():
    if "nc" not in _cached:
        _cached["nc"] = build_nc()
    return _cached["nc"]


def _make_runner(nc, n_cores):
    """One-time construction of a reusable jitted SPMD executor (the stock
    run_bass_kernel_spmd path rebuilds + retraces the jax function on every
    call, ~280ms of host overhead per invocation)."""
    import jax
    from jax.sharding import Mesh, PartitionSpec
    from concourse import bass2jax
    from concourse import mybir as _mb

    shard_map = bass2jax.shard_map

    bass2jax.install_neuronx_cc_hook()
    partition_name = (
        nc.partition_id_tensor.name if nc.partition_id_tensor else None
    )
    in_names, out_names, out_avals = [], [], []
    for alloc in nc.m.functions[0].allocations:
        if not isinstance(alloc, _mb.MemoryLocationSet):
            continue
        name = alloc.memorylocations[0].name
        if alloc.kind == "ExternalInput":
            if name != partition_name:
                in_names.append(name)
        elif alloc.kind == "ExternalOutput":
            out_names.append(name)
            out_avals.append(jax.core.ShapedArray(
                tuple(alloc.tensor_shape), _mb.dt.np(alloc.dtype)))
    n_params = len(in_names)
    all_names = in_names + out_names
    if partition_name is not None:
        all_names = all_names + [partition_name]
    all_names = tuple(all_names)

    def _body(*args):
        operands = list(args)
        if partition_name is not None:
            operands.append(bass2jax.partition_id_tensor())
        outs = bass2jax._bass_exec_p.bind(
            *operands,
            out_avals=tuple(out_avals),
            in_names=all_names,
            out_names=tuple(out_names),
            lowering_input_output_aliases=(),
            sim_require_finite=True,
            sim_require_nnan=True,
            nc=nc,
        )
        return tuple(outs)

    devices = jax.devices()[:n_cores]
    mesh = Mesh(np.asarray(devices), ("core",))
    n_outs = len(out_names)
    sharded = jax.jit(
        shard_map(
            _body, mesh=mesh,
            in_specs=(PartitionSpec("core"),) * (n_params + n_outs),
            out_specs=(PartitionSpec("core"),) * n_outs,
            check_rep=False,
        ),
        donate_argnums=tuple(range(n_params, n_params + n_outs)),
        keep_unused=True,
    )

    def run(in_maps):
        concat_in = [
            np.concatenate([np.asarray(m[name]) for m in in_maps], axis=0)
            for name in in_names
        ]
        concat_zeros = [
            np.zeros((n_cores * a.shape[0], *a.shape[1:]), a.dtype)
            for a in out_avals
        ]
        out_arrs = sharded(*concat_in, *concat_zeros)
        return [
            {
                name: np.asarray(out_arrs[i]).reshape(
                    n_cores, *out_avals[i].shape)[c]
                for i, name in enumerate(out_names)
            }
            for c in range(n_cores)
        ]

    return run


def _get_runner():
    if "runner" not in _cached:
        _cached["runner"] = _make_runner(_get_nc(), NIB * NJC)
    return _cached["runner"]


def kernel(R, box):
    R = np.asarray(R, np.float32)
    box = np.asarray(box, np.float32)
    assert R.shape == (N, 3)
    assert np.allclose(box, np.eye(3, dtype=np.float32) * BOX_L), (
        "kernel compiled for box = 20*I"
    )
    in_maps = host_prep(R)
    results = _get_runner()(in_maps)
    partials = [results[c]["out"] for c in range(NIB * NJC)]
    return host_combine(partials)



# revision 39
# speedup vs baseline: 1.0022x; 1.0022x over previous
"""Trainium2 Bass kernel for the N^3 triplet descriptor (gnn_message_passing).

Strategy: the reference's O(N^3) angular sum factorizes exactly via the
Legendre addition theorem into O(N^2) per-pair vector moments:

  P0 term: (sum_j w_j)^2
  P1 term: |sum_j w_j u_j|^2                  (u = unit displacement)
  P2 term: 1.5*|sum_j w_j u_j u_j^T|_F^2 - 0.5*(sum_j w_j)^2

with w_j = fc(r_ij) * r_ij^n.  Each device accumulates 36 pair moments per
central atom (9 radial powers, 9 S1 components, 9+9 symmetric S2
components); the tiny nonlinear combine runs on host after gathering.

Sharding: 8 cores = 2 i-blocks (96 rows on partitions) x 4 j-chunks (48
neighbors on the free axis). Cross-j-chunk partials are summed on host.

v2 engine plan (per core):
- PE matmul broadcasts the pair geometry: u[p,(d,j)] = Rj[d,j]-Ri[p,d]
  from a tiny [4,240] input (ones row + Ri^T | Rj^T, -I pattern), so the
  input DMA drops from 61KB to 3.8KB and starts the pipeline ~1.3us
  earlier.
- DVE: minimum image via two comparisons + adds reading PSUM directly
  (ALU mod is rejected by walrus codegen; ACT Sign corrupts execution in
  full programs - hardware-bisected), deg-4 Horner for fc (refit;
  end-to-end rel err 5e-4 vs the 2e-2 gate), fp32 radial chain with
  r,r2,r4 adjacent so e3..e10 build in 3 strided ops, bf16 angular
  products in DVE 2x_1p mode (half cost).
- ACT: sq = Square(dx) fp32 and sqb = Square(dx)->bf16, rinv via the
  abs_reciprocal_sqrt table (single table set, loaded once at t=0 by a
  dummy activation).
- GpSimd: consts, cross products poffb, m25 mask, r (x2), r4, rinv2.
  GpSimd tensor work only overlaps contiguous DVE phases (concurrent
  GpSimd ops slow strided DVE ops ~4x via SBUF ports).
"""

import numpy as np

import concourse.bass as bass
import concourse.bacc as bacc
from concourse import mybir
from concourse.bass_utils import run_bass_kernel_spmd

F32 = mybir.dt.float32
BF16 = mybir.dt.bfloat16
ALU = mybir.AluOpType
ACT = mybir.ActivationFunctionType

N = 192
NI = 96          # i rows per core (partition dim)
NJ = 48          # j neighbors per core (free dim)
NIB = 2          # i blocks
NJC = 4          # j chunks
BOX_L = 20.0
RC = 5.0
FC_DEG = 4   # deg-4 fit err 4.2e-5 -> end-to-end 5e-4 (gate is 2e-2)
R2_EPS = 1e-12

# fc(w) = 0.5*(1+cos(pi*sqrt(w)/RC)) as poly in w = r^2, w in [0, RC^2]
_FC_W = np.linspace(0, RC * RC, 20001)
_FC_Y = 0.5 * (1 + np.cos(np.pi * np.sqrt(_FC_W) / RC))
_FC_C = (
    np.polynomial.chebyshev.Chebyshev.fit(_FC_W, _FC_Y, FC_DEG, domain=[0, RC * RC])
    .convert(kind=np.polynomial.Polynomial)
    .coef.astype(np.float64)
)

_cached = {}


def _v(ap, off, dims):
    """Custom free-dim view of an SBUF tile AP: keep partition dim, replace
    free dims, shift offset by `off` elements."""
    return bass.AP(ap.tensor, ap.offset + off, [list(ap.ap[0])] + [list(d) for d in dims])


def build_nc():
    # Suppress the Bass.__init__ const-pool preamble (4 gpsimd memsets + an
    # all-engine barrier, ~0.8us of kernel startup): this kernel registers
    # its own const tiles (c_eps, c_zero) set inside the block.
    _orig_barrier = bass.Bass.all_engine_barrier
    _orig_memset = bass.BassSharedVectorInterface.memset
    bass.Bass.all_engine_barrier = lambda self: None
    bass.BassSharedVectorInterface.memset = lambda self, ap, v: None
    try:
        nc = bacc.Bacc(
            "TRN2",
            target_bir_lowering=False,
            debug=False,
            enable_asserts=True,
            num_devices=NIB * NJC,
        )
    finally:
        bass.Bass.all_engine_barrier = _orig_barrier
        bass.BassSharedVectorInterface.memset = _orig_memset

    in_d = nc.dram_tensor("geom", [4, 240], F32, kind="ExternalInput").ap()
    out_d = nc.dram_tensor("out", [NI, 36], F32, kind="ExternalOutput").ap()

    in_s = nc.alloc_sbuf_tensor("in_s", [4, 240], F32).ap()
    u_ps = nc.alloc_psum_tensor("u_ps", [NI, 144], F32).ap()
    geo = nc.alloc_sbuf_tensor("geo", [NI, 288], F32).ap()    # dx | sq (d-major)
    mic = nc.alloc_sbuf_tensor("mic", [NI, 288], F32).ap()    # t1/t2 | xm
    dxb = nc.alloc_sbuf_tensor("dxb", [NI, 144], BF16).ap()
    geo2 = nc.alloc_sbuf_tensor("geo2", [NI, 288], BF16).ap() # sqb | poffb
    rv = nc.alloc_sbuf_tensor("rv", [NI, 144], F32).ap()      # r | r2 | r4
    rvp = nc.alloc_sbuf_tensor("rvp", [NI, 192], F32).ap()    # rinv2|rinv|one|r
    m25 = nc.alloc_sbuf_tensor("m25", [NI, NJ], F32).ap()
    yh = nc.alloc_sbuf_tensor("yh", [NI, NJ], F32).ap()
    wx = nc.alloc_sbuf_tensor("wx", [NI, 9 * NJ], F32).ap()   # e2..e10
    bq = nc.alloc_sbuf_tensor("bq", [NI, 4 * NJ], BF16).ap()  # b0..b3
    big3 = nc.alloc_sbuf_tensor("big3", [NI, 1296], BF16).ap()
    sg = nc.alloc_sbuf_tensor("sg", [NI, 36], F32).ap()
    scr = nc.alloc_sbuf_tensor("scr", [1, 8], F32).ap()
    c_eps = nc.alloc_sbuf_tensor("c_eps", [128, 1], F32).ap()
    c_zero = nc.alloc_sbuf_tensor("c_zero", [128, 1], F32).ap()
    nc.const_aps.aps[(F32, R2_EPS)] = c_eps
    nc.const_aps.aps[(F32, 0.0)] = c_zero

    dsem = nc.alloc_semaphore("dsem")
    vq = nc.alloc_semaphore("vq")      # DVE instruction counter
    sqm = nc.alloc_semaphore("sqm")    # ACT instruction counter
    gq = nc.alloc_semaphore("gq")      # GpSimd instruction counter
    pq = nc.alloc_semaphore("pq")      # PE counter

    dx = geo[:, 0:144]
    sq = geo[:, 144:288]
    sqb = geo2[:, 0:144]
    tc = mic[:, 0:144]
    xm = mic[:, 144:288]
    rr = rv[:, 0:NJ]
    r2 = rv[:, NJ:2 * NJ]
    r4 = rv[:, 2 * NJ:3 * NJ]
    rinv2 = rvp[:, 0:NJ]
    rinv = rvp[:, NJ:2 * NJ]
    onep = rvp[:, 2 * NJ:3 * NJ]
    rr2 = rvp[:, 3 * NJ:4 * NJ]
    fc = wx[:, 0:NJ]                   # e2 = fc * r^0

    c = [float(x) for x in _FC_C]

    # cross-engine wait points (per-engine instruction-counter values)
    VQ_DX = 4
    VQ_DXB = 5
    VQ_R2 = 6
    VQ_FC = 7 + FC_DEG       # fc done (Horner = FC_DEG+1 ops)
    VQ_QR = VQ_FC + 4        # radial moments in sg[0:9]
    VQ_S2 = VQ_QR + 3        # S2 products in big3
    VQ_RED1 = VQ_S2 + 1      # sg[9:27] done
    VQ_RED2 = VQ_S2 + 2      # sg[27:36] done
    SQ_SQ = 2                # sq ready
    SQ_RINV = 3
    SQ_SQB = 4
    GQ_EPS = 1
    GQ_CONST = 2             # c_eps + c_zero
    GQ_POFF = 5
    GQ_M25 = 6
    GQ_R = 7
    GQ_R4 = 9
    GQ_RINV2 = 10

    with nc.Block() as block:

        @block.sync
        def _(sync):
            sync.dma_start(in_s, in_d).then_inc(dsem, 16)
            sync.wait_ge(vq, VQ_QR)
            sync.dma_start(out_d[:, 0:9], sg[:, 0:9], single_packet=True).then_inc(dsem, 16)
            sync.wait_ge(vq, VQ_RED1)
            sync.dma_start(out_d[:, 9:27], sg[:, 9:27], single_packet=True).then_inc(dsem, 16)
            sync.wait_ge(dsem, 64)

        @block.tensor
        def _(tensor):
            tensor.wait_ge(dsem, 16)
            # u[p,(d,j)] = 1*Rj[d,j] + sum_d' Ri[p,d']*(-delta)
            tensor.matmul(
                u_ps, in_s[:, 0:96], in_s[:, 96:240], start=True, stop=True,
            ).then_inc(pq, 1)

        @block.scalar
        def _(scalar):
            sn = [0]

            def S(inst):
                if sn[0] > 0:
                    inst._wait_ge(sqm, sn[0])
                inst.then_inc(sqm, 1)
                sn[0] += 1
                return inst

            # dummy activation: pulls the single ACT table set load
            # (abs_reciprocal_sqrt_and_small; has square/identity) to t=0,
            # overlapped with the input DMA + PE matmul
            scalar.wait_ge(gq, GQ_EPS)
            S(scalar.activation(
                scr[0:1, 0:1], c_eps[0:1, :], ACT.Abs_reciprocal_sqrt,
                bias=R2_EPS))
            scalar.wait_ge(gq, GQ_CONST)
            scalar.wait_ge(vq, VQ_DX)
            S(scalar.activation(sq, dx, ACT.Square, bias=0.0))
            scalar.wait_ge(vq, VQ_R2)
            # rinv = 1/sqrt(r2 + eps)
            S(scalar.activation(rinv, r2, ACT.Abs_reciprocal_sqrt, bias=R2_EPS))
            assert sn[0] == SQ_RINV
            S(scalar.activation(sqb, dx, ACT.Square, bias=0.0))
            assert sn[0] == SQ_SQB
            # final 9-col out-DMA from the idle Scalar HWDGE queue
            scalar.wait_ge(vq, VQ_RED2)
            scalar.dma_start(out_d[:, 27:36], sg[:, 27:36], single_packet=True).then_inc(dsem, 16)

        @block.gpsimd
        def _(gpsimd):
            gn = [0]

            def G(inst):
                if gn[0] > 0:
                    inst._wait_ge(gq, gn[0])
                inst.then_inc(gq, 1)
                gn[0] += 1
                return inst

            G(gpsimd.memset(c_eps, R2_EPS))
            G(gpsimd.memset(c_zero, 0.0))
            G(gpsimd.memset(onep, 1.0))
            # cross products for S2 off-diagonal, bf16 (angular-only)
            gpsimd.wait_ge(vq, VQ_DXB)
            G(gpsimd.tensor_tensor(
                geo2[:, 144:240], dxb[:, 0:96], dxb[:, 48:144], op=ALU.mult))
            G(gpsimd.tensor_tensor(
                geo2[:, 240:288], dxb[:, 0:NJ], dxb[:, 96:144], op=ALU.mult))
            assert gn[0] == GQ_POFF
            gpsimd.wait_ge(vq, VQ_R2)
            G(gpsimd.tensor_scalar(m25, r2, RC * RC, None, op0=ALU.is_lt))
            assert gn[0] == GQ_M25
            gpsimd.wait_ge(sqm, SQ_RINV)
            G(gpsimd.tensor_tensor(rr, r2, rinv, op=ALU.mult))
            G(gpsimd.tensor_tensor(rr2, r2, rinv, op=ALU.mult))
            G(gpsimd.tensor_tensor(r4, r2, r2, op=ALU.mult))
            G(gpsimd.tensor_tensor(rinv2, rinv, rinv, op=ALU.mult))
            assert gn[0] == GQ_RINV2

        @block.vector
        def _(vector):
            vn = [0]

            def V(inst):
                if vn[0] > 0:
                    inst._wait_ge(vq, vn[0])
                inst.then_inc(vq, 1)
                vn[0] += 1
                return inst

            vector.wait_ge(pq, 1)
            # minimum image: dx = u - 20*(u>=10) + 20*(u<-10)
            V(vector.tensor_scalar(
                tc, u_ps, BOX_L / 2, -BOX_L, op0=ALU.is_ge, op1=ALU.mult))
            V(vector.tensor_tensor(xm, u_ps, tc, op=ALU.add))
            V(vector.tensor_scalar(
                tc, u_ps, -BOX_L / 2, BOX_L, op0=ALU.is_lt, op1=ALU.mult))
            V(vector.tensor_tensor(dx, xm, tc, op=ALU.add))
            assert vn[0] == VQ_DX
            V(vector.tensor_copy(dxb, dx))
            assert vn[0] == VQ_DXB
            vector.wait_ge(sqm, SQ_SQ)
            V(vector.tensor_reduce(
                r2, _v(geo, 144, [[1, NJ], [NJ, 3]]),
                axis=mybir.AxisListType.X, op=ALU.add,
            ))
            assert vn[0] == VQ_R2
            # fc = poly(r2) * (r2 < RC^2), Horner on DVE (m25 from GpSimd)
            V(vector.tensor_scalar(yh, r2, c[FC_DEG], None, op0=ALU.mult))
            for k in range(FC_DEG - 1, 0, -1):
                V(vector.scalar_tensor_tensor(
                    yh, yh, c[k], r2, op0=ALU.add, op1=ALU.mult))
            vector.wait_ge(gq, GQ_M25)
            V(vector.scalar_tensor_tensor(
                fc, yh, c[0], m25, op0=ALU.add, op1=ALU.mult))
            assert vn[0] == VQ_FC
            # radial family e_k = fc * r^(k-2), fp32: [e3|e4] = fc*[r|r2],
            # [e5|e6] = [e3|e4]*r2, [e7..e10] = [e3..e6]*r4
            vector.wait_ge(gq, GQ_R)
            V(vector.tensor_tensor(
                _v(wx, NJ, [[NJ, 2], [1, NJ]]),
                _v(wx, 0, [[0, 2], [1, NJ]]),
                _v(rv, 0, [[NJ, 2], [1, NJ]]),
                op=ALU.mult))
            V(vector.tensor_tensor(
                _v(wx, 3 * NJ, [[NJ, 2], [1, NJ]]),
                _v(wx, NJ, [[NJ, 2], [1, NJ]]),
                _v(rv, NJ, [[0, 2], [1, NJ]]),
                op=ALU.mult))
            vector.wait_ge(gq, GQ_R4)
            V(vector.tensor_tensor(
                _v(wx, 5 * NJ, [[NJ, 4], [1, NJ]]),
                _v(wx, NJ, [[NJ, 4], [1, NJ]]),
                _v(rv, 2 * NJ, [[0, 4], [1, NJ]]),
                op=ALU.mult))
            # radial moments: q_r[k] = sum_j e_{k+2}
            V(vector.tensor_reduce(
                sg[:, 0:9], _v(wx, 0, [[NJ, 9], [1, NJ]]),
                axis=mybir.AxisListType.X, op=ALU.add,
            ))
            assert vn[0] == VQ_QR
            # bf16 angular weights b_k = fc * r^(k-2), k=0..3, one strided op
            vector.wait_ge(gq, GQ_RINV2)
            V(vector.tensor_tensor(
                _v(bq, 0, [[NJ, 4], [1, NJ]]),
                _v(wx, 0, [[0, 4], [1, NJ]]),
                _v(rvp, 0, [[NJ, 4], [1, NJ]]),
                op=ALU.mult))
            # S1 products: big3[n,d,j] = b_{n+1} * dxb_d   (bf16 2x mode)
            V(vector.tensor_tensor(
                _v(big3, 0, [[144, 3], [NJ, 3], [1, NJ]]),
                _v(bq, NJ, [[NJ, 3], [0, 3], [1, NJ]]),
                _v(dxb, 0, [[0, 3], [NJ, 3], [1, NJ]]),
                op=ALU.mult))
            # S2 products: big3[n,hd,j] = b_n * geo2_hd    (bf16 2x mode)
            vector.wait_ge(sqm, SQ_SQB)
            V(vector.tensor_tensor(
                _v(big3, 432, [[288, 3], [NJ, 6], [1, NJ]]),
                _v(bq, 0, [[NJ, 3], [0, 6], [1, NJ]]),
                _v(geo2, 0, [[0, 3], [NJ, 6], [1, NJ]]),
                op=ALU.mult))
            assert vn[0] == VQ_S2
            V(vector.tensor_reduce(
                sg[:, 9:27], _v(big3, 0, [[NJ, 18], [1, NJ]]),
                axis=mybir.AxisListType.X, op=ALU.add,
            ))
            assert vn[0] == VQ_RED1
            V(vector.tensor_reduce(
                sg[:, 27:36], _v(big3, 864, [[NJ, 9], [1, NJ]]),
                axis=mybir.AxisListType.X, op=ALU.add,
            ))
            assert vn[0] == VQ_RED2, vn[0]

    nc.compile()
    return nc


def host_prep(R):
    """Per-core input arrays: [4, 240] = [ones;Ri^T | Rj^T, -I]."""
    R = np.ascontiguousarray(R, np.float32)
    in_maps = []
    for core in range(NIB * NJC):
        ib, jc = divmod(core, NJC)
        g = np.zeros((4, 240), np.float32)
        ri = R[ib * NI:(ib + 1) * NI, :]              # [96, 3]
        rj = R[jc * NJ:(jc + 1) * NJ, :]              # [48, 3]
        g[0, 0:96] = 1.0
        g[1:4, 0:96] = ri.T
        g[0, 96:240] = rj.T.reshape(144)              # d-major
        for d in range(3):
            g[1 + d, 96 + d * NJ:96 + (d + 1) * NJ] = -1.0
        in_maps.append({"geom": g})
    return in_maps


def host_combine(partials):
    """partials: list of 8 [96,36] arrays (core order). Returns [192,18]."""
    sums = np.zeros((N, 36), np.float64)
    for core, p in enumerate(partials):
        ib = core // NJC
        sums[ib * NI:(ib + 1) * NI] += p.astype(np.float64)
    q_r = sums[:, 0:9].astype(np.float32)
    q_r[:, 0] -= np.float32(_FC_C[0])                 # remove j==i self term
    s0 = q_r[:, 0:3].astype(np.float64)               # [N,3] n=0..2
    s1 = sums[:, 9:18].reshape(N, 3, 3)               # [N,n,d]
    s2 = sums[:, 18:36].reshape(N, 3, 6)              # [N,n,(sq_xyz,po_xyz)]
    s2d = s2[:, :, 0:3]
    s2o = s2[:, :, 3:6]
    ang = np.empty((N, 3, 3), np.float64)
    ang[:, :, 0] = s0 * s0
    ang[:, :, 1] = (s1 * s1).sum(-1)
    fro2 = (s2d * s2d).sum(-1) + 2.0 * (s2o * s2o).sum(-1)
    ang[:, :, 2] = 1.5 * fro2 - 0.5 * s0 * s0
    return np.concatenate(
        [q_r, ang.reshape(N, 9).astype(np.float32)], axis=-1)


def _get_nc():
    if "nc" not in _cached:
        _cached["nc"] = build_nc()
    return _cached["nc"]


def _make_runner(nc, n_cores):
    """One-time construction of a reusable jitted SPMD executor (the stock
    run_bass_kernel_spmd path rebuilds + retraces the jax function on every
    call, ~280ms of host overhead per invocation)."""
    import jax
    from jax.sharding import Mesh, PartitionSpec
    from concourse import bass2jax
    from concourse import mybir as _mb

    shard_map = bass2jax.shard_map

    bass2jax.install_neuronx_cc_hook()
    partition_name = (
        nc.partition_id_tensor.name if nc.partition_id_tensor else None
    )
    in_names, out_names, out_avals = [], [], []
    for alloc in nc.m.functions[0].allocations:
        if not isinstance(alloc, _mb.MemoryLocationSet):
            continue
        name = alloc.memorylocations[0].name
        if alloc.kind == "ExternalInput":
            if name != partition_name:
                in_names.append(name)
        elif alloc.kind == "ExternalOutput":
            out_names.append(name)
            out_avals.append(jax.core.ShapedArray(
                tuple(alloc.tensor_shape), _mb.dt.np(alloc.dtype)))
    n_params = len(in_names)
    all_names = in_names + out_names
    if partition_name is not None:
        all_names = all_names + [partition_name]
    all_names = tuple(all_names)

    def _body(*args):
        operands = list(args)
        if partition_name is not None:
            operands.append(bass2jax.partition_id_tensor())
        outs = bass2jax._bass_exec_p.bind(
            *operands,
            out_avals=tuple(out_avals),
            in_names=all_names,
            out_names=tuple(out_names),
            lowering_input_output_aliases=(),
            sim_require_finite=True,
            sim_require_nnan=True,
            nc=nc,
        )
        return tuple(outs)

    devices = jax.devices()[:n_cores]
    mesh = Mesh(np.asarray(devices), ("core",))
    n_outs = len(out_names)
    sharded = jax.jit(
        shard_map(
            _body, mesh=mesh,
            in_specs=(PartitionSpec("core"),) * (n_params + n_outs),
            out_specs=(PartitionSpec("core"),) * n_outs,
            check_rep=False,
        ),
        donate_argnums=tuple(range(n_params, n_params + n_outs)),
        keep_unused=True,
    )

    def run(in_maps):
        concat_in = [
            np.concatenate([np.asarray(m[name]) for m in in_maps], axis=0)
            for name in in_names
        ]
        concat_zeros = [
            np.zeros((n_cores * a.shape[0], *a.shape[1:]), a.dtype)
            for a in out_avals
        ]
        out_arrs = sharded(*concat_in, *concat_zeros)
        return [
            {
                name: np.asarray(out_arrs[i]).reshape(
                    n_cores, *out_avals[i].shape)[c]
                for i, name in enumerate(out_names)
            }
            for c in range(n_cores)
        ]

    return run


def _get_runner():
    if "runner" not in _cached:
        _cached["runner"] = _make_runner(_get_nc(), NIB * NJC)
    return _cached["runner"]


def kernel(R, box):
    R = np.asarray(R, np.float32)
    box = np.asarray(box, np.float32)
    assert R.shape == (N, 3)
    assert np.allclose(box, np.eye(3, dtype=np.float32) * BOX_L), (
        "kernel compiled for box = 20*I"
    )
    in_maps = host_prep(R)
    results = _get_runner()(in_maps)
    partials = [results[c]["out"] for c in range(NIB * NJC)]
    return host_combine(partials)


# revision 48
# speedup vs baseline: 1.0478x; 1.0455x over previous
"""Trainium2 Bass kernel for the N^3 triplet descriptor (gnn_message_passing).

Strategy: the reference's O(N^3) angular sum factorizes exactly via the
Legendre addition theorem into O(N^2) per-pair vector moments:

  P0 term: (sum_j w_j)^2
  P1 term: |sum_j w_j u_j|^2                  (u = unit displacement)
  P2 term: 1.5*|sum_j w_j u_j u_j^T|_F^2 - 0.5*(sum_j w_j)^2

with w_j = fc(r_ij) * r_ij^n.  Each device accumulates 36 pair moments per
central atom (9 radial powers, 9 S1 components, 9+9 symmetric S2
components); the tiny nonlinear combine runs on host after gathering.

Sharding: 8 cores = 2 i-blocks (96 rows on partitions) x 4 j-chunks (48
neighbors on the free axis). Cross-j-chunk partials are summed on host.

v3 engine plan (per core), tuned from hardware traces:
- Input DMA split across the scalar+vector HWDGE queues, both issued at
  the very top of the program (the sync queue drains ~700ns at start; a
  PE-matmul input broadcast was tried and lost ~800ns to the two-pass
  fp32 matmul + drain - DMA completion latency ~1.4us is size-invariant).
- DVE: minimum image via two comparisons + adds (ALU mod is rejected by
  walrus codegen; ACT Sign corrupts execution in full programs -
  hardware-bisected), m25 mask inline (GpSimd tensor_scalar measured
  895ns vs 182ns on DVE), deg-4 Horner for fc (end-to-end rel err 5e-4
  vs the 2e-2 gate), fp32 radial chain with r,r2,r4 adjacent so e3..e10
  build in 3 strided ops, bf16 angular products in DVE 2x_1p mode.
- ACT: sq = Square(dx), rinv via abs_reciprocal_sqrt (one table set,
  pulled to t=0 by a dummy), sqb/dxb bf16 copies of the geometry.
- GpSimd: consts, poffb cross products, r (x2), r4, rinv2, and the S2
  tail summed as an fp32-state cumulative scan in parallel with DVE's
  big reduce (GpSimd tensor_reduce can't do free-axis; the scan's
  per-group running totals are differenced on host - linear, so summing
  cores first is fine).
"""

import numpy as np

import concourse.bass as bass
import concourse.bacc as bacc
from concourse import mybir
from concourse.bass_utils import run_bass_kernel_spmd

F32 = mybir.dt.float32
BF16 = mybir.dt.bfloat16
ALU = mybir.AluOpType
ACT = mybir.ActivationFunctionType

N = 192
NI = 96          # i rows per core (partition dim)
NJ = 48          # j neighbors per core (free dim)
NIB = 2          # i blocks
NJC = 4          # j chunks
BOX_L = 20.0
RC = 5.0
FC_DEG = 4   # deg-4 fit err 4.2e-5 -> end-to-end 5e-4 (gate is 2e-2)
R2_EPS = 1e-12

# fc(w) = 0.5*(1+cos(pi*sqrt(w)/RC)) as poly in w = r^2, w in [0, RC^2]
_FC_W = np.linspace(0, RC * RC, 20001)
_FC_Y = 0.5 * (1 + np.cos(np.pi * np.sqrt(_FC_W) / RC))
_FC_C = (
    np.polynomial.chebyshev.Chebyshev.fit(_FC_W, _FC_Y, FC_DEG, domain=[0, RC * RC])
    .convert(kind=np.polynomial.Polynomial)
    .coef.astype(np.float64)
)

_cached = {}


def _v(ap, off, dims):
    """Custom free-dim view of an SBUF tile AP: keep partition dim, replace
    free dims, shift offset by `off` elements."""
    return bass.AP(ap.tensor, ap.offset + off, [list(ap.ap[0])] + [list(d) for d in dims])


def build_nc():
    # Suppress the Bass.__init__ const-pool preamble (4 gpsimd memsets + an
    # all-engine barrier, ~0.8us of kernel startup): this kernel registers
    # its own const tiles (c_eps, c_zero) set inside the block.
    _orig_barrier = bass.Bass.all_engine_barrier
    _orig_memset = bass.BassSharedVectorInterface.memset
    bass.Bass.all_engine_barrier = lambda self: None
    bass.BassSharedVectorInterface.memset = lambda self, ap, v: None
    try:
        nc = bacc.Bacc(
            "TRN2",
            target_bir_lowering=False,
            debug=False,
            enable_asserts=True,
            num_devices=NIB * NJC,
        )
    finally:
        bass.Bass.all_engine_barrier = _orig_barrier
        bass.BassSharedVectorInterface.memset = _orig_memset

    in_d = nc.dram_tensor("rji", [NI, 160], F32, kind="ExternalInput").ap()
    out_d = nc.dram_tensor("out", [NI, 36], F32, kind="ExternalOutput").ap()

    rji = nc.alloc_sbuf_tensor("rji_s", [NI, 160], F32).ap()
    geo = nc.alloc_sbuf_tensor("geo", [NI, 288], F32).ap()    # dx | sq (d-major)
    mic = nc.alloc_sbuf_tensor("mic", [NI, 432], F32).ap()    # dxr | tc | xm
    dxb = nc.alloc_sbuf_tensor("dxb", [NI, 144], BF16).ap()
    geo2 = nc.alloc_sbuf_tensor("geo2", [NI, 288], BF16).ap() # sqb | poffb
    rv = nc.alloc_sbuf_tensor("rv", [NI, 144], F32).ap()      # r | r2 | r4
    rvp = nc.alloc_sbuf_tensor("rvp", [NI, 192], F32).ap()    # rinv2|rinv|one|r
    m25 = nc.alloc_sbuf_tensor("m25", [NI, NJ], F32).ap()
    yh = nc.alloc_sbuf_tensor("yh", [NI, NJ], F32).ap()
    wx = nc.alloc_sbuf_tensor("wx", [NI, 9 * NJ], F32).ap()   # e2..e10
    bq = nc.alloc_sbuf_tensor("bq", [NI, 4 * NJ], BF16).ap()  # b0..b3
    big3 = nc.alloc_sbuf_tensor("big3", [NI, 1296], BF16).ap()
    cum = nc.alloc_sbuf_tensor("cum", [NI, 432], F32).ap()    # S2-tail scan
    sg = nc.alloc_sbuf_tensor("sg", [NI, 36], F32).ap()
    scr = nc.alloc_sbuf_tensor("scr", [1, 8], F32).ap()
    c_eps = nc.alloc_sbuf_tensor("c_eps", [128, 1], F32).ap()
    c_zero = nc.alloc_sbuf_tensor("c_zero", [128, 1], F32).ap()
    nc.const_aps.aps[(F32, R2_EPS)] = c_eps
    nc.const_aps.aps[(F32, 0.0)] = c_zero

    dsem = nc.alloc_semaphore("dsem")
    vq = nc.alloc_semaphore("vq")      # DVE instruction counter
    sqm = nc.alloc_semaphore("sqm")    # ACT instruction counter
    gq = nc.alloc_semaphore("gq")      # GpSimd instruction counter

    dx = geo[:, 0:144]
    sq = geo[:, 144:288]
    sqb = geo2[:, 0:144]
    dxr = mic[:, 0:144]
    tc = mic[:, 144:288]
    xm = mic[:, 288:432]
    rr = rv[:, 0:NJ]
    r2 = rv[:, NJ:2 * NJ]
    r4 = rv[:, 2 * NJ:3 * NJ]
    rinv2 = rvp[:, 0:NJ]
    rinv = rvp[:, NJ:2 * NJ]
    onep = rvp[:, 2 * NJ:3 * NJ]
    rr2 = rvp[:, 3 * NJ:4 * NJ]
    fc = wx[:, 0:NJ]                   # e2 = fc * r^0

    rj3 = rji[:, 0:144].rearrange("p (d j) -> p d j", d=3)
    ri3 = rji[:, 144:147].unsqueeze(-1).broadcast_to((NI, 3, NJ))
    dxr3 = dxr.rearrange("p (d j) -> p d j", d=3)

    c = [float(x) for x in _FC_C]

    # cross-engine wait points (per-engine instruction-counter values)
    VQ_DX = 5
    VQ_R2 = 6
    VQ_FC = 8 + FC_DEG       # fc done (m25 + Horner FC_DEG+1 ops)
    VQ_QR = VQ_FC + 4        # radial moments in sg[0:9]
    VQ_S2 = VQ_QR + 3        # S2 products in big3
    VQ_RED1 = VQ_S2 + 1      # sg[9:27] done
    VQ_RED2 = VQ_S2 + 2      # sg[27:36] done
    SQ_SQ = 2                # sq ready
    SQ_RINV = 3
    SQ_SQB = 4
    SQ_DXB = 5
    GQ_EPS = 1
    GQ_CONST = 2             # c_eps + c_zero
    GQ_POFF = 5
    GQ_R = 6
    GQ_R4 = 8
    GQ_RINV2 = 9
    GQ_SCAN = 10

    with nc.Block() as block:

        @block.sync
        def _(sync):
            sync.wait_ge(vq, VQ_QR)
            sync.dma_start(out_d[:, 0:9], sg[:, 0:9], single_packet=True).then_inc(dsem, 16)
            sync.wait_ge(vq, VQ_RED1)
            sync.dma_start(out_d[:, 9:27], sg[:, 9:27], single_packet=True).then_inc(dsem, 16)
            sync.wait_ge(dsem, 80)

        @block.scalar
        def _(scalar):
            sn = [0]

            def S(inst):
                if sn[0] > 0:
                    inst._wait_ge(sqm, sn[0])
                inst.then_inc(sqm, 1)
                sn[0] += 1
                return inst

            # first input-DMA half: desc-gen on the scalar HWDGE queue runs
            # in parallel with the ACT table load below
            scalar.dma_start(rji[:, 0:80], in_d[:, 0:80]).then_inc(dsem, 16)
            # dummy activation: pulls the single ACT table set load
            # (abs_reciprocal_sqrt_and_small; has square/copy) to t=0
            scalar.wait_ge(gq, GQ_EPS)
            S(scalar.activation(
                scr[0:1, 0:1], c_eps[0:1, :], ACT.Abs_reciprocal_sqrt,
                bias=R2_EPS))
            scalar.wait_ge(gq, GQ_CONST)
            scalar.wait_ge(vq, VQ_DX)
            S(scalar.activation(sq, dx, ACT.Square, bias=0.0))
            scalar.wait_ge(vq, VQ_R2)
            # rinv = 1/sqrt(r2 + eps)
            S(scalar.activation(rinv, r2, ACT.Abs_reciprocal_sqrt, bias=R2_EPS))
            assert sn[0] == SQ_RINV
            # bf16 geometry for the angular products (ACT is idle here)
            S(scalar.activation(sqb, dx, ACT.Square, bias=0.0))
            assert sn[0] == SQ_SQB
            S(scalar.activation(dxb, dx, ACT.Copy, bias=0.0))
            assert sn[0] == SQ_DXB
            # S2-tail out-DMA from the idle Scalar HWDGE queue
            scalar.wait_ge(vq, VQ_RED2)
            scalar.dma_start(
                out_d[:, 27:36], sg[:, 27:36],
                single_packet=True).then_inc(dsem, 16)

        @block.gpsimd
        def _(gpsimd):
            gn = [0]

            def G(inst):
                if gn[0] > 0:
                    inst._wait_ge(gq, gn[0])
                inst.then_inc(gq, 1)
                gn[0] += 1
                return inst

            G(gpsimd.memset(c_eps, R2_EPS))
            # second input-DMA half on the gpsimd HWDGE queue (sync's
            # queue drains ~700ns at program start; DVE can't issue DMAs)
            gpsimd.dma_start(rji[:, 80:160], in_d[:, 80:160]).then_inc(dsem, 16)
            G(gpsimd.memset(c_zero, 0.0))
            G(gpsimd.memset(onep, 1.0))
            # cross products for S2 off-diagonal, bf16 out (angular-only)
            gpsimd.wait_ge(vq, VQ_DX)
            G(gpsimd.tensor_tensor(
                geo2[:, 144:240], dx[:, 0:96], dx[:, 48:144], op=ALU.mult))
            G(gpsimd.tensor_tensor(
                geo2[:, 240:288], dx[:, 0:NJ], dx[:, 96:144], op=ALU.mult))
            assert gn[0] == GQ_POFF
            gpsimd.wait_ge(sqm, SQ_RINV)
            G(gpsimd.tensor_tensor(rr, r2, rinv, op=ALU.mult))
            G(gpsimd.tensor_tensor(rr2, r2, rinv, op=ALU.mult))
            G(gpsimd.tensor_tensor(r4, r2, r2, op=ALU.mult))
            G(gpsimd.tensor_tensor(rinv2, rinv, rinv, op=ALU.mult))
            assert gn[0] == GQ_RINV2

        @block.vector
        def _(vector):
            vn = [0]

            def V(inst):
                if vn[0] > 0:
                    inst._wait_ge(vq, vn[0])
                inst.then_inc(vq, 1)
                vn[0] += 1
                return inst

            vector.wait_ge(dsem, 32)
            V(vector.tensor_tensor(dxr3, rj3, ri3, op=ALU.subtract))
            # minimum image: dx = dxr - 20*(dxr>=10) + 20*(dxr<-10)
            V(vector.tensor_scalar(
                tc, dxr, BOX_L / 2, -BOX_L, op0=ALU.is_ge, op1=ALU.mult))
            V(vector.tensor_tensor(xm, dxr, tc, op=ALU.add))
            V(vector.tensor_scalar(
                tc, dxr, -BOX_L / 2, BOX_L, op0=ALU.is_lt, op1=ALU.mult))
            V(vector.tensor_tensor(dx, xm, tc, op=ALU.add))
            assert vn[0] == VQ_DX
            vector.wait_ge(sqm, SQ_SQ)
            V(vector.tensor_reduce(
                r2, _v(geo, 144, [[1, NJ], [NJ, 3]]),
                axis=mybir.AxisListType.X, op=ALU.add,
            ))
            assert vn[0] == VQ_R2
            # fc = poly(r2) * (r2 < RC^2), all on DVE
            V(vector.tensor_scalar(m25, r2, RC * RC, None, op0=ALU.is_lt))
            V(vector.tensor_scalar(yh, r2, c[FC_DEG], None, op0=ALU.mult))
            for k in range(FC_DEG - 1, 0, -1):
                V(vector.scalar_tensor_tensor(
                    yh, yh, c[k], r2, op0=ALU.add, op1=ALU.mult))
            V(vector.scalar_tensor_tensor(
                fc, yh, c[0], m25, op0=ALU.add, op1=ALU.mult))
            assert vn[0] == VQ_FC
            # radial family e_k = fc * r^(k-2), fp32: [e3|e4] = fc*[r|r2],
            # [e5|e6] = [e3|e4]*r2, [e7..e10] = [e3..e6]*r4
            vector.wait_ge(gq, GQ_R)
            V(vector.tensor_tensor(
                _v(wx, NJ, [[NJ, 2], [1, NJ]]),
                _v(wx, 0, [[0, 2], [1, NJ]]),
                _v(rv, 0, [[NJ, 2], [1, NJ]]),
                op=ALU.mult))
            V(vector.tensor_tensor(
                _v(wx, 3 * NJ, [[NJ, 2], [1, NJ]]),
                _v(wx, NJ, [[NJ, 2], [1, NJ]]),
                _v(rv, NJ, [[0, 2], [1, NJ]]),
                op=ALU.mult))
            vector.wait_ge(gq, GQ_R4)
            V(vector.tensor_tensor(
                _v(wx, 5 * NJ, [[NJ, 4], [1, NJ]]),
                _v(wx, NJ, [[NJ, 4], [1, NJ]]),
                _v(rv, 2 * NJ, [[0, 4], [1, NJ]]),
                op=ALU.mult))
            # radial moments: q_r[k] = sum_j e_{k+2}
            V(vector.tensor_reduce(
                sg[:, 0:9], _v(wx, 0, [[NJ, 9], [1, NJ]]),
                axis=mybir.AxisListType.X, op=ALU.add,
            ))
            assert vn[0] == VQ_QR
            # bf16 angular weights b_k = fc * r^(k-2), k=0..3, one strided op
            vector.wait_ge(gq, GQ_RINV2)
            V(vector.tensor_tensor(
                _v(bq, 0, [[NJ, 4], [1, NJ]]),
                _v(wx, 0, [[0, 4], [1, NJ]]),
                _v(rvp, 0, [[NJ, 4], [1, NJ]]),
                op=ALU.mult))
            # S1 products: big3[n,d,j] = b_{n+1} * dxb_d   (bf16 2x mode)
            vector.wait_ge(sqm, SQ_DXB)
            V(vector.tensor_tensor(
                _v(big3, 0, [[144, 3], [NJ, 3], [1, NJ]]),
                _v(bq, NJ, [[NJ, 3], [0, 3], [1, NJ]]),
                _v(dxb, 0, [[0, 3], [NJ, 3], [1, NJ]]),
                op=ALU.mult))
            # S2 products: big3[n,hd,j] = b_n * geo2_hd    (bf16 2x mode)
            V(vector.tensor_tensor(
                _v(big3, 432, [[288, 3], [NJ, 6], [1, NJ]]),
                _v(bq, 0, [[NJ, 3], [0, 6], [1, NJ]]),
                _v(geo2, 0, [[0, 3], [NJ, 6], [1, NJ]]),
                op=ALU.mult))
            assert vn[0] == VQ_S2
            V(vector.tensor_reduce(
                sg[:, 9:27], _v(big3, 0, [[NJ, 18], [1, NJ]]),
                axis=mybir.AxisListType.X, op=ALU.add,
            ))
            assert vn[0] == VQ_RED1
            V(vector.tensor_reduce(
                sg[:, 27:36], _v(big3, 864, [[NJ, 9], [1, NJ]]),
                axis=mybir.AxisListType.X, op=ALU.add,
            ))
            assert vn[0] == VQ_RED2, vn[0]

    nc.compile()
    return nc


def host_prep(R):
    """Per-core input arrays: [96, 160] = [RjT replicated | Ri | pad]."""
    R = np.ascontiguousarray(R, np.float32)
    in_maps = []
    for core in range(NIB * NJC):
        ib, jc = divmod(core, NJC)
        rji = np.zeros((NI, 160), np.float32)
        rj = R[jc * NJ:(jc + 1) * NJ, :]              # [48, 3]
        rji[:, 0:144] = rj.T.reshape(1, 144)          # d-major, replicated
        rji[:, 144:147] = R[ib * NI:(ib + 1) * NI, :]
        in_maps.append({"rji": rji})
    return in_maps


def host_combine(partials):
    """partials: list of 8 [96,36] arrays (core order). Returns [192,18]."""
    sums = np.zeros((N, 36), np.float64)
    for core, p in enumerate(partials):
        ib = core // NJC
        sums[ib * NI:(ib + 1) * NI] += p.astype(np.float64)
    q_r = sums[:, 0:9].astype(np.float32)
    q_r[:, 0] -= np.float32(_FC_C[0])                 # remove j==i self term
    s0 = q_r[:, 0:3].astype(np.float64)               # [N,3] n=0..2
    s1 = sums[:, 9:18].reshape(N, 3, 3)               # [N,n,d]
    s2 = sums[:, 18:36].reshape(N, 3, 6)              # [N,n,(sq_xyz,po_xyz)]
    s2d = s2[:, :, 0:3]
    s2o = s2[:, :, 3:6]
    ang = np.empty((N, 3, 3), np.float64)
    ang[:, :, 0] = s0 * s0
    ang[:, :, 1] = (s1 * s1).sum(-1)
    fro2 = (s2d * s2d).sum(-1) + 2.0 * (s2o * s2o).sum(-1)
    ang[:, :, 2] = 1.5 * fro2 - 0.5 * s0 * s0
    return np.concatenate(
        [q_r, ang.reshape(N, 9).astype(np.float32)], axis=-1)


def _get_nc():
    if "nc" not in _cached:
        _cached["nc"] = build_nc()
    return _cached["nc"]


def _make_runner(nc, n_cores):
    """One-time construction of a reusable jitted SPMD executor (the stock
    run_bass_kernel_spmd path rebuilds + retraces the jax function on every
    call, ~280ms of host overhead per invocation)."""
    import jax
    from jax.sharding import Mesh, PartitionSpec
    from concourse import bass2jax
    from concourse import mybir as _mb

    shard_map = bass2jax.shard_map

    bass2jax.install_neuronx_cc_hook()
    partition_name = (
        nc.partition_id_tensor.name if nc.partition_id_tensor else None
    )
    in_names, out_names, out_avals = [], [], []
    for alloc in nc.m.functions[0].allocations:
        if not isinstance(alloc, _mb.MemoryLocationSet):
            continue
        name = alloc.memorylocations[0].name
        if alloc.kind == "ExternalInput":
            if name != partition_name:
                in_names.append(name)
        elif alloc.kind == "ExternalOutput":
            out_names.append(name)
            out_avals.append(jax.core.ShapedArray(
                tuple(alloc.tensor_shape), _mb.dt.np(alloc.dtype)))
    n_params = len(in_names)
    all_names = in_names + out_names
    if partition_name is not None:
        all_names = all_names + [partition_name]
    all_names = tuple(all_names)

    def _body(*args):
        operands = list(args)
        if partition_name is not None:
            operands.append(bass2jax.partition_id_tensor())
        outs = bass2jax._bass_exec_p.bind(
            *operands,
            out_avals=tuple(out_avals),
            in_names=all_names,
            out_names=tuple(out_names),
            lowering_input_output_aliases=(),
            sim_require_finite=True,
            sim_require_nnan=True,
            nc=nc,
        )
        return tuple(outs)

    devices = jax.devices()[:n_cores]
    mesh = Mesh(np.asarray(devices), ("core",))
    n_outs = len(out_names)
    sharded = jax.jit(
        shard_map(
            _body, mesh=mesh,
            in_specs=(PartitionSpec("core"),) * (n_params + n_outs),
            out_specs=(PartitionSpec("core"),) * n_outs,
            check_rep=False,
        ),
        donate_argnums=tuple(range(n_params, n_params + n_outs)),
        keep_unused=True,
    )

    def run(in_maps):
        concat_in = [
            np.concatenate([np.asarray(m[name]) for m in in_maps], axis=0)
            for name in in_names
        ]
        concat_zeros = [
            np.zeros((n_cores * a.shape[0], *a.shape[1:]), a.dtype)
            for a in out_avals
        ]
        out_arrs = sharded(*concat_in, *concat_zeros)
        return [
            {
                name: np.asarray(out_arrs[i]).reshape(
                    n_cores, *out_avals[i].shape)[c]
                for i, name in enumerate(out_names)
            }
            for c in range(n_cores)
        ]

    return run


def _get_runner():
    if "runner" not in _cached:
        _cached["runner"] = _make_runner(_get_nc(), NIB * NJC)
    return _cached["runner"]


def kernel(R, box):
    R = np.asarray(R, np.float32)
    box = np.asarray(box, np.float32)
    assert R.shape == (N, 3)
    assert np.allclose(box, np.eye(3, dtype=np.float32) * BOX_L), (
        "kernel compiled for box = 20*I"
    )
    in_maps = host_prep(R)
    results = _get_runner()(in_maps)
    partials = [results[c]["out"] for c in range(NIB * NJC)]
    return host_combine(partials)


# revision 49
# speedup vs baseline: 1.1172x; 1.0663x over previous
"""Trainium2 Bass kernel for the N^3 triplet descriptor (gnn_message_passing).

Strategy: the reference's O(N^3) angular sum factorizes exactly via the
Legendre addition theorem into O(N^2) per-pair vector moments:

  P0 term: (sum_j w_j)^2
  P1 term: |sum_j w_j u_j|^2                  (u = unit displacement)
  P2 term: 1.5*|sum_j w_j u_j u_j^T|_F^2 - 0.5*(sum_j w_j)^2

with w_j = fc(r_ij) * r_ij^n.  Each device accumulates 36 pair moments per
central atom (9 radial powers, 9 S1 components, 9+9 symmetric S2
components); the tiny nonlinear combine runs on host after gathering.

Sharding: 8 cores = 2 i-blocks (96 rows on partitions) x 4 j-chunks (48
neighbors on the free axis). Cross-j-chunk partials are summed on host.

v4 engine plan (per core), tuned from hardware traces:
- One input DMA on the scalar HWDGE queue, issued first (the sync queue
  drains ~700ns at start; gpsimd's queue adds ~900ns; a PE-matmul input
  broadcast lost ~800ns to the two-pass fp32 matmul + drain - DMA
  completion latency ~1.4us is size-invariant).
- DVE: minimum image via two comparisons + adds (ALU mod is rejected by
  walrus codegen; ACT Sign corrupts execution in full programs -
  hardware-bisected), sq/r2 inline (an ACT round trip costs more than
  the 305ns DVE op), deg-4 Horner for fc, then a bf16 weight family:
  b0..b3 and e4..e10 share one bf16 tile so the radial e-chain runs in
  2x_1p mode; the radial reduce accumulates bf16 planes in fp32
  (end-to-end rel err 2.4e-3 vs the 2e-2 gate, numpy-validated).
  Angular products bf16 2x, one merged 27-group angular reduce.
- ACT: rinv via abs_reciprocal_sqrt (one table set, pulled to t=0 by a
  dummy), r2b/sqb/dxb bf16 copies of the geometry while DVE runs Horner.
- GpSimd: consts, poffb cross products, r, rinv2, r4b.
"""

import numpy as np

import concourse.bass as bass
import concourse.bacc as bacc
from concourse import mybir
from concourse.bass_utils import run_bass_kernel_spmd

F32 = mybir.dt.float32
BF16 = mybir.dt.bfloat16
ALU = mybir.AluOpType
ACT = mybir.ActivationFunctionType

N = 192
NI = 96          # i rows per core (partition dim)
NJ = 48          # j neighbors per core (free dim)
NIB = 2          # i blocks
NJC = 4          # j chunks
BOX_L = 20.0
RC = 5.0
FC_DEG = 4   # deg-4 fit err 4.2e-5 -> end-to-end 5e-4 (gate is 2e-2)
R2_EPS = 1e-12

# fc(w) = 0.5*(1+cos(pi*sqrt(w)/RC)) as poly in w = r^2, w in [0, RC^2]
_FC_W = np.linspace(0, RC * RC, 20001)
_FC_Y = 0.5 * (1 + np.cos(np.pi * np.sqrt(_FC_W) / RC))
_FC_C = (
    np.polynomial.chebyshev.Chebyshev.fit(_FC_W, _FC_Y, FC_DEG, domain=[0, RC * RC])
    .convert(kind=np.polynomial.Polynomial)
    .coef.astype(np.float64)
)

_cached = {}


def _v(ap, off, dims):
    """Custom free-dim view of an SBUF tile AP: keep partition dim, replace
    free dims, shift offset by `off` elements."""
    return bass.AP(ap.tensor, ap.offset + off, [list(ap.ap[0])] + [list(d) for d in dims])


def build_nc():
    # Suppress the Bass.__init__ const-pool preamble (4 gpsimd memsets + an
    # all-engine barrier, ~0.8us of kernel startup): this kernel registers
    # its own const tiles (c_eps, c_zero) set inside the block.
    _orig_barrier = bass.Bass.all_engine_barrier
    _orig_memset = bass.BassSharedVectorInterface.memset
    bass.Bass.all_engine_barrier = lambda self: None
    bass.BassSharedVectorInterface.memset = lambda self, ap, v: None
    try:
        nc = bacc.Bacc(
            "TRN2",
            target_bir_lowering=False,
            debug=False,
            enable_asserts=True,
            num_devices=NIB * NJC,
        )
    finally:
        bass.Bass.all_engine_barrier = _orig_barrier
        bass.BassSharedVectorInterface.memset = _orig_memset

    in_d = nc.dram_tensor("rji", [NI, 160], F32, kind="ExternalInput").ap()
    out_d = nc.dram_tensor("out", [NI, 36], F32, kind="ExternalOutput").ap()

    rji = nc.alloc_sbuf_tensor("rji_s", [NI, 160], F32).ap()
    geo = nc.alloc_sbuf_tensor("geo", [NI, 288], F32).ap()    # dx | sq (d-major)
    mic = nc.alloc_sbuf_tensor("mic", [NI, 432], F32).ap()    # dxr | tc | xm
    dxb = nc.alloc_sbuf_tensor("dxb", [NI, 144], BF16).ap()
    geo2 = nc.alloc_sbuf_tensor("geo2", [NI, 288], BF16).ap() # sqb | poffb
    rv = nc.alloc_sbuf_tensor("rv", [NI, NJ], F32).ap()       # r2
    rvb = nc.alloc_sbuf_tensor("rvb", [NI, 2 * NJ], BF16).ap()  # r2b | r4b
    rvp = nc.alloc_sbuf_tensor("rvp", [NI, 192], F32).ap()    # rinv2|rinv|one|r
    m25 = nc.alloc_sbuf_tensor("m25", [NI, NJ], F32).ap()
    yh = nc.alloc_sbuf_tensor("yh", [NI, NJ], F32).ap()
    bqx = nc.alloc_sbuf_tensor("bqx", [NI, 11 * NJ], BF16).ap()  # b0..b3|e4..e10
    big3 = nc.alloc_sbuf_tensor("big3", [NI, 1296], BF16).ap()
    sg = nc.alloc_sbuf_tensor("sg", [NI, 36], F32).ap()
    scr = nc.alloc_sbuf_tensor("scr", [1, 8], F32).ap()
    c_eps = nc.alloc_sbuf_tensor("c_eps", [128, 1], F32).ap()
    c_zero = nc.alloc_sbuf_tensor("c_zero", [128, 1], F32).ap()
    nc.const_aps.aps[(F32, R2_EPS)] = c_eps
    nc.const_aps.aps[(F32, 0.0)] = c_zero

    dsem = nc.alloc_semaphore("dsem")
    vq = nc.alloc_semaphore("vq")      # DVE instruction counter
    sqm = nc.alloc_semaphore("sqm")    # ACT instruction counter
    gq = nc.alloc_semaphore("gq")      # GpSimd instruction counter

    dx = geo[:, 0:144]
    sq = geo[:, 144:288]
    sqb = geo2[:, 0:144]
    dxr = mic[:, 0:144]
    tc = mic[:, 144:288]
    xm = mic[:, 288:432]
    r2 = rv[:, 0:NJ]
    r2b = rvb[:, 0:NJ]
    r4b = rvb[:, NJ:2 * NJ]
    rinv2 = rvp[:, 0:NJ]
    rinv = rvp[:, NJ:2 * NJ]
    onep = rvp[:, 2 * NJ:3 * NJ]
    rr = rvp[:, 3 * NJ:4 * NJ]
    fc = yh                            # Horner result stays in yh

    rj3 = rji[:, 0:144].rearrange("p (d j) -> p d j", d=3)
    ri3 = rji[:, 144:147].unsqueeze(-1).broadcast_to((NI, 3, NJ))
    dxr3 = dxr.rearrange("p (d j) -> p d j", d=3)

    c = [float(x) for x in _FC_C]

    # cross-engine wait points (per-engine instruction-counter values)
    VQ_DX = 5
    VQ_R2 = 7
    VQ_FC = 9 + FC_DEG       # fc done (m25 + Horner FC_DEG+1 ops)
    VQ_QR = VQ_FC + 5        # radial moments in sg[0:9]
    VQ_S2 = VQ_QR + 2        # S2 products in big3
    VQ_RED = VQ_S2 + 1       # sg[9:36] done
    SQ_RINV = 2
    SQ_R2B = 3
    SQ_SQB = 4
    SQ_DXB = 5
    GQ_EPS = 1
    GQ_CONST = 2             # c_eps + c_zero
    GQ_POFF = 5
    GQ_R = 6
    GQ_RINV2 = 7
    GQ_R4B = 8

    with nc.Block() as block:

        @block.sync
        def _(sync):
            sync.wait_ge(vq, VQ_QR)
            sync.dma_start(out_d[:, 0:9], sg[:, 0:9], single_packet=True).then_inc(dsem, 16)
            sync.wait_ge(vq, VQ_RED)
            sync.dma_start(out_d[:, 9:36], sg[:, 9:36], single_packet=True).then_inc(dsem, 16)
            sync.wait_ge(dsem, 48)

        @block.scalar
        def _(scalar):
            sn = [0]

            def S(inst):
                if sn[0] > 0:
                    inst._wait_ge(sqm, sn[0])
                inst.then_inc(sqm, 1)
                sn[0] += 1
                return inst

            # input DMA: desc-gen on the scalar HWDGE queue runs in
            # parallel with the ACT table load below
            scalar.dma_start(rji, in_d).then_inc(dsem, 16)
            # dummy activation: pulls the single ACT table set load
            # (abs_reciprocal_sqrt_and_small; has square/copy) to t=0
            scalar.wait_ge(gq, GQ_EPS)
            S(scalar.activation(
                scr[0:1, 0:1], c_eps[0:1, :], ACT.Abs_reciprocal_sqrt,
                bias=R2_EPS))
            scalar.wait_ge(gq, GQ_CONST)
            scalar.wait_ge(vq, VQ_R2)
            # rinv = 1/sqrt(r2 + eps)
            S(scalar.activation(rinv, r2, ACT.Abs_reciprocal_sqrt, bias=R2_EPS))
            assert sn[0] == SQ_RINV
            # bf16 geometry for the radial/angular chains while DVE Horners
            S(scalar.activation(r2b, r2, ACT.Copy, bias=0.0))
            assert sn[0] == SQ_R2B
            S(scalar.activation(sqb, dx, ACT.Square, bias=0.0))
            assert sn[0] == SQ_SQB
            S(scalar.activation(dxb, dx, ACT.Copy, bias=0.0))
            assert sn[0] == SQ_DXB

        @block.gpsimd
        def _(gpsimd):
            gn = [0]

            def G(inst):
                if gn[0] > 0:
                    inst._wait_ge(gq, gn[0])
                inst.then_inc(gq, 1)
                gn[0] += 1
                return inst

            G(gpsimd.memset(c_eps, R2_EPS))
            G(gpsimd.memset(c_zero, 0.0))
            G(gpsimd.memset(onep, 1.0))
            # cross products for S2 off-diagonal, bf16 out (angular-only)
            gpsimd.wait_ge(vq, VQ_DX)
            G(gpsimd.tensor_tensor(
                geo2[:, 144:240], dx[:, 0:96], dx[:, 48:144], op=ALU.mult))
            G(gpsimd.tensor_tensor(
                geo2[:, 240:288], dx[:, 0:NJ], dx[:, 96:144], op=ALU.mult))
            assert gn[0] == GQ_POFF
            gpsimd.wait_ge(sqm, SQ_RINV)
            G(gpsimd.tensor_tensor(rr, r2, rinv, op=ALU.mult))
            G(gpsimd.tensor_tensor(rinv2, rinv, rinv, op=ALU.mult))
            G(gpsimd.tensor_tensor(r4b, r2, r2, op=ALU.mult))
            assert gn[0] == GQ_R4B

        @block.vector
        def _(vector):
            vn = [0]

            def V(inst):
                if vn[0] > 0:
                    inst._wait_ge(vq, vn[0])
                inst.then_inc(vq, 1)
                vn[0] += 1
                return inst

            vector.wait_ge(dsem, 16)
            V(vector.tensor_tensor(dxr3, rj3, ri3, op=ALU.subtract))
            # minimum image: dx = dxr - 20*(dxr>=10) + 20*(dxr<-10)
            V(vector.tensor_scalar(
                tc, dxr, BOX_L / 2, -BOX_L, op0=ALU.is_ge, op1=ALU.mult))
            V(vector.tensor_tensor(xm, dxr, tc, op=ALU.add))
            V(vector.tensor_scalar(
                tc, dxr, -BOX_L / 2, BOX_L, op0=ALU.is_lt, op1=ALU.mult))
            V(vector.tensor_tensor(dx, xm, tc, op=ALU.add))
            assert vn[0] == VQ_DX
            V(vector.tensor_tensor(sq, dx, dx, op=ALU.mult))
            V(vector.tensor_reduce(
                r2, _v(geo, 144, [[1, NJ], [NJ, 3]]),
                axis=mybir.AxisListType.X, op=ALU.add,
            ))
            assert vn[0] == VQ_R2
            # fc = poly(r2) * (r2 < RC^2), all on DVE
            V(vector.tensor_scalar(m25, r2, RC * RC, None, op0=ALU.is_lt))
            V(vector.tensor_scalar(yh, r2, c[FC_DEG], None, op0=ALU.mult))
            for k in range(FC_DEG - 1, 0, -1):
                V(vector.scalar_tensor_tensor(
                    yh, yh, c[k], r2, op0=ALU.add, op1=ALU.mult))
            V(vector.scalar_tensor_tensor(
                fc, yh, c[0], m25, op0=ALU.add, op1=ALU.mult))
            assert vn[0] == VQ_FC
            # bf16 weight family: [b0..b3] = fc * [1/r2|1/r|1|r] (fp32 in,
            # bf16 out), then the radial tail in 2x bf16:
            # [e4|e5] = [b2|b3]*r2b, [e6..e9] = [e2..e5]*r4b, e10 = e6*r4b
            vector.wait_ge(gq, GQ_RINV2)
            V(vector.tensor_tensor(
                _v(bqx, 0, [[NJ, 4], [1, NJ]]),
                _v(yh, 0, [[0, 4], [1, NJ]]),
                _v(rvp, 0, [[NJ, 4], [1, NJ]]),
                op=ALU.mult))
            vector.wait_ge(sqm, SQ_R2B)
            V(vector.tensor_tensor(
                _v(bqx, 4 * NJ, [[NJ, 2], [1, NJ]]),
                _v(bqx, 2 * NJ, [[NJ, 2], [1, NJ]]),
                _v(rvb, 0, [[0, 2], [1, NJ]]),
                op=ALU.mult))
            vector.wait_ge(gq, GQ_R4B)
            V(vector.tensor_tensor(
                _v(bqx, 6 * NJ, [[NJ, 4], [1, NJ]]),
                _v(bqx, 2 * NJ, [[NJ, 4], [1, NJ]]),
                _v(rvb, NJ, [[0, 4], [1, NJ]]),
                op=ALU.mult))
            V(vector.tensor_tensor(
                bqx[:, 10 * NJ:11 * NJ], bqx[:, 6 * NJ:7 * NJ], r4b,
                op=ALU.mult))
            # radial moments: q_r[k] = sum_j e_{k+2} (fp32 accumulate)
            V(vector.tensor_reduce(
                sg[:, 0:9], _v(bqx, 2 * NJ, [[NJ, 9], [1, NJ]]),
                axis=mybir.AxisListType.X, op=ALU.add,
            ))
            assert vn[0] == VQ_QR
            # S1 products: big3[n,d,j] = b_{n+1} * dxb_d   (bf16 2x mode)
            vector.wait_ge(sqm, SQ_DXB)
            V(vector.tensor_tensor(
                _v(big3, 0, [[144, 3], [NJ, 3], [1, NJ]]),
                _v(bqx, NJ, [[NJ, 3], [0, 3], [1, NJ]]),
                _v(dxb, 0, [[0, 3], [NJ, 3], [1, NJ]]),
                op=ALU.mult))
            # S2 products: big3[n,hd,j] = b_n * geo2_hd    (bf16 2x mode)
            V(vector.tensor_tensor(
                _v(big3, 432, [[288, 3], [NJ, 6], [1, NJ]]),
                _v(bqx, 0, [[NJ, 3], [0, 6], [1, NJ]]),
                _v(geo2, 0, [[0, 3], [NJ, 6], [1, NJ]]),
                op=ALU.mult))
            assert vn[0] == VQ_S2
            V(vector.tensor_reduce(
                sg[:, 9:36], _v(big3, 0, [[NJ, 27], [1, NJ]]),
                axis=mybir.AxisListType.X, op=ALU.add,
            ))
            assert vn[0] == VQ_RED, vn[0]

    nc.compile()
    return nc


def host_prep(R):
    """Per-core input arrays: [96, 160] = [RjT replicated | Ri | pad]."""
    R = np.ascontiguousarray(R, np.float32)
    in_maps = []
    for core in range(NIB * NJC):
        ib, jc = divmod(core, NJC)
        rji = np.zeros((NI, 160), np.float32)
        rj = R[jc * NJ:(jc + 1) * NJ, :]              # [48, 3]
        rji[:, 0:144] = rj.T.reshape(1, 144)          # d-major, replicated
        rji[:, 144:147] = R[ib * NI:(ib + 1) * NI, :]
        in_maps.append({"rji": rji})
    return in_maps


def host_combine(partials):
    """partials: list of 8 [96,36] arrays (core order). Returns [192,18]."""
    sums = np.zeros((N, 36), np.float64)
    for core, p in enumerate(partials):
        ib = core // NJC
        sums[ib * NI:(ib + 1) * NI] += p.astype(np.float64)
    q_r = sums[:, 0:9].astype(np.float32)
    q_r[:, 0] -= np.float32(_FC_C[0])                 # remove j==i self term
    s0 = q_r[:, 0:3].astype(np.float64)               # [N,3] n=0..2
    s1 = sums[:, 9:18].reshape(N, 3, 3)               # [N,n,d]
    s2 = sums[:, 18:36].reshape(N, 3, 6)              # [N,n,(sq_xyz,po_xyz)]
    s2d = s2[:, :, 0:3]
    s2o = s2[:, :, 3:6]
    ang = np.empty((N, 3, 3), np.float64)
    ang[:, :, 0] = s0 * s0
    ang[:, :, 1] = (s1 * s1).sum(-1)
    fro2 = (s2d * s2d).sum(-1) + 2.0 * (s2o * s2o).sum(-1)
    ang[:, :, 2] = 1.5 * fro2 - 0.5 * s0 * s0
    return np.concatenate(
        [q_r, ang.reshape(N, 9).astype(np.float32)], axis=-1)


def _get_nc():
    if "nc" not in _cached:
        _cached["nc"] = build_nc()
    return _cached["nc"]


def _make_runner(nc, n_cores):
    """One-time construction of a reusable jitted SPMD executor (the stock
    run_bass_kernel_spmd path rebuilds + retraces the jax function on every
    call, ~280ms of host overhead per invocation)."""
    import jax
    from jax.sharding import Mesh, PartitionSpec
    from concourse import bass2jax
    from concourse import mybir as _mb

    shard_map = bass2jax.shard_map

    bass2jax.install_neuronx_cc_hook()
    partition_name = (
        nc.partition_id_tensor.name if nc.partition_id_tensor else None
    )
    in_names, out_names, out_avals = [], [], []
    for alloc in nc.m.functions[0].allocations:
        if not isinstance(alloc, _mb.MemoryLocationSet):
            continue
        name = alloc.memorylocations[0].name
        if alloc.kind == "ExternalInput":
            if name != partition_name:
                in_names.append(name)
        elif alloc.kind == "ExternalOutput":
            out_names.append(name)
            out_avals.append(jax.core.ShapedArray(
                tuple(alloc.tensor_shape), _mb.dt.np(alloc.dtype)))
    n_params = len(in_names)
    all_names = in_names + out_names
    if partition_name is not None:
        all_names = all_names + [partition_name]
    all_names = tuple(all_names)

    def _body(*args):
        operands = list(args)
        if partition_name is not None:
            operands.append(bass2jax.partition_id_tensor())
        outs = bass2jax._bass_exec_p.bind(
            *operands,
            out_avals=tuple(out_avals),
            in_names=all_names,
            out_names=tuple(out_names),
            lowering_input_output_aliases=(),
            sim_require_finite=True,
            sim_require_nnan=True,
            nc=nc,
        )
        return tuple(outs)

    devices = jax.devices()[:n_cores]
    mesh = Mesh(np.asarray(devices), ("core",))
    n_outs = len(out_names)
    sharded = jax.jit(
        shard_map(
            _body, mesh=mesh,
            in_specs=(PartitionSpec("core"),) * (n_params + n_outs),
            out_specs=(PartitionSpec("core"),) * n_outs,
            check_rep=False,
        ),
        donate_argnums=tuple(range(n_params, n_params + n_outs)),
        keep_unused=True,
    )

    def run(in_maps):
        concat_in = [
            np.concatenate([np.asarray(m[name]) for m in in_maps], axis=0)
            for name in in_names
        ]
        concat_zeros = [
            np.zeros((n_cores * a.shape[0], *a.shape[1:]), a.dtype)
            for a in out_avals
        ]
        out_arrs = sharded(*concat_in, *concat_zeros)
        return [
            {
                name: np.asarray(out_arrs[i]).reshape(
                    n_cores, *out_avals[i].shape)[c]
                for i, name in enumerate(out_names)
            }
            for c in range(n_cores)
        ]

    return run


def _get_runner():
    if "runner" not in _cached:
        _cached["runner"] = _make_runner(_get_nc(), NIB * NJC)
    return _cached["runner"]


def kernel(R, box):
    R = np.asarray(R, np.float32)
    box = np.asarray(box, np.float32)
    assert R.shape == (N, 3)
    assert np.allclose(box, np.eye(3, dtype=np.float32) * BOX_L), (
        "kernel compiled for box = 20*I"
    )
    in_maps = host_prep(R)
    results = _get_runner()(in_maps)
    partials = [results[c]["out"] for c in range(NIB * NJC)]
    return host_combine(partials)
